# revision 15
# baseline (speedup 1.0000x reference)
"""ArcFace loss on 8 TRN2 NeuronCores, tensor-parallel over the class dim.

Reference computation (B=1024, D=512, C=100000):
    e = l2norm(embeddings); w = l2norm(weight)
    cos = clip(e @ w.T);  phi = cos(theta + m) with easy-margin fallback
    logits = S * (onehot*phi + (1-onehot)*cos);  loss = mean CE

Distribution: classes sharded 12500/core. Host pre-normalizes both e and w
(row L2) and ships fp8-e4m3 copies for the big matmul plus fp32 copies of
the normalized embeddings / label-weight rows for the exact target-logit
path. Each core computes its partial sum-of-exp Z_b over its class shard:
fp8 DoubleRow matmuls (2 per 128-batch x 512-class tile, 256 contraction
rows per instruction) into PSUM, then per-tile exp with a fused
per-partition row-sum. No softmax max-shift is needed: |cos|<=1 so
S*cos in [-64, 64] and exp() stays comfortably inside fp32 range.

    Z[b]   = sum_c exp(S*cos[b,c])          (allgathered + summed)
    nll[b] = log(Z - exp(S*cos_t) + exp(S*phi)) - S*phi
    loss   = mean_b nll[b]

The exp work is split across engines so the Scalar engine stays ahead of
the PE: 5 of 8 batch tiles per chunk use the exact Scalar-engine Exp
(accum_out fusion), the other 3 use a Schraudolph bit-trick exp on the
Vector engine (int32 mad + bitcast reinterpret, ~2.5%/element with a
near-zero-mean magic constant) reduced in a second DVE pass. Z is a
12.5k-term sum, so the per-element noise washes out far inside the loss
tolerance.
"""

import math

import numpy as np
import ml_dtypes

import concourse.bass as bass
import concourse.bass_isa as bass_isa
import concourse.tile as tile
from concourse import bacc, mybir
from concourse.bass_utils import run_bass_kernel_spmd

# problem shapes (hardcoded per spec)
B, D, C = 1024, 512, 100000
N_CORES = 8
CS = C // N_CORES            # 12500 classes per core
NBT = B // 128               # 8 batch tiles
NKT = D // 128               # 4 contraction tiles
CHUNK = 512                  # matmul free-dim chunk
N_CHUNKS = (CS + CHUNK - 1) // CHUNK   # 25 (last chunk 212 wide)

# arcface constants
S = 64.0
M = 0.5
COS_M = math.cos(M)
SIN_M = math.sin(M)
TH = math.cos(math.pi - M)
MM_ = math.sin(math.pi - M) * M
EPS = 1e-7

F32 = mybir.dt.float32
I32 = mybir.dt.int32
FP8 = mybir.dt.float8e4

# Schraudolph exp-approx constants (folding in the S logit scale):
#   exp(S*x) ~= bitcast_f32(int32(SCHRA_A * x + SCHRA_B))
SCHRA_A = S * (1 << 23) / math.log(2.0)
SCHRA_B = float(127 * (1 << 23) - 486411)   # near-zero-mean magic constant
DVE_B = (2, 5, 7)                           # batch tiles handled by DVE

_NC_CACHE = []


def _emit_body(nc, pools, params, zc_in, zc_out):
    """Emit one full kernel body. Shared with the unrolled timing bench."""
    singles, work, wtp, tiny, psump = pools
    wt, et, er, wlab, out_ext = params

    # ---- replicated fp8 embedding tiles (stationary operands) ----
    et_s = singles.tile([128, NKT, B], FP8, name="et_s")
    for k in range(NKT):
        nc.sync.dma_start(out=et_s[:, k, :], in_=et[k * 128:(k + 1) * 128, :])

    dt_ = tiny.tile([128, NBT], F32, name="dt_")  # cos to target = e_n.wlab_n
    zcols = singles.tile([128, NBT, N_CHUNKS], F32, name="zcols")
    st = tiny.tile([128, NBT], F32, name="st")    # S * phi  (target logit)
    ect = tiny.tile([128, NBT], F32, name="ect")  # exp(S * cos_t)
    ept = tiny.tile([128, NBT], F32, name="ept")  # exp(S * phi)

    def tgt_dot(b):
        # one [128, D] fp32 dot-product tile of the target path
        er_t = work.tile([128, D], F32, name="er_t")
        nc.sync.dma_start(out=er_t, in_=er[b * 128:(b + 1) * 128, :])
        wl_t = work.tile([128, D], F32, name="wl_t")
        nc.sync.dma_start(out=wl_t, in_=wlab[b * 128:(b + 1) * 128, :])
        prod = work.tile([128, D], F32, name="prod")
        nc.vector.scalar_tensor_tensor(
            out=prod, in0=er_t, scalar=1.0, in1=wl_t,
            op0=mybir.AluOpType.mult, op1=mybir.AluOpType.mult,
            accum_out=dt_[:, b:b + 1])

    def tgt_phi():
        # margin transform on the [128, NBT] target-cos tile
        cost = tiny.tile([128, NBT], F32, name="cost")
        nc.vector.tensor_scalar_min(cost, dt_, 1.0 - EPS)
        nc.vector.tensor_scalar_max(cost, cost, -1.0 + EPS)
        c2 = tiny.tile([128, NBT], F32, name="c2")
        nc.vector.tensor_mul(c2, cost, cost)
        sint = tiny.tile([128, NBT], F32, name="sint")   # sqrt(1 - cos^2)
        nc.scalar.activation(out=sint, in_=c2,
                             func=mybir.ActivationFunctionType.Sqrt,
                             bias=1.0, scale=-1.0)
        pa = tiny.tile([128, NBT], F32, name="pa")
        nc.vector.tensor_scalar_mul(pa, cost, COS_M)
        pb = tiny.tile([128, NBT], F32, name="pb")
        nc.vector.tensor_scalar_mul(pb, sint, SIN_M)
        phi = tiny.tile([128, NBT], F32, name="phi")
        nc.vector.tensor_sub(phi, pa, pb)
        msk = tiny.tile([128, NBT], F32, name="msk")
        nc.vector.tensor_scalar(out=msk, in0=cost, scalar1=TH,
                                scalar2=None, op0=mybir.AluOpType.is_gt)
        alt = tiny.tile([128, NBT], F32, name="alt")
        nc.vector.tensor_scalar_sub(alt, cost, MM_)
        dd = tiny.tile([128, NBT], F32, name="dd")
        nc.vector.tensor_sub(dd, phi, alt)
        md = tiny.tile([128, NBT], F32, name="md")
        nc.vector.tensor_mul(md, msk, dd)
        phif = tiny.tile([128, NBT], F32, name="phif")  # where(cos>TH, ...)
        nc.vector.tensor_add(phif, alt, md)
        nc.vector.tensor_scalar_mul(st, phif, S)
        nc.scalar.activation(out=ect, in_=dt_,
                             func=mybir.ActivationFunctionType.Exp, scale=S)
        nc.scalar.activation(out=ept, in_=st,
                             func=mybir.ActivationFunctionType.Exp)

    # ---- main pipeline over class chunks ----
    for ct in range(N_CHUNKS):
        c0 = ct * CHUNK
        cw = min(CHUNK, CS - c0)           # 512 or 212

        wt8c = wtp.tile([128, NKT, CHUNK], FP8, name="wt8c")
        for k in range(NKT):
            nc.sync.dma_start(out=wt8c[:, k, :cw],
                              in_=wt[k * 128:(k + 1) * 128, c0:c0 + cw])

        for b in range(NBT):
            ps = psump.tile([128, CHUNK], F32, name="ps")
            nc.tensor.matmul(
                out=ps[:, :cw],
                lhsT=et_s[:, 0:2, b * 128:(b + 1) * 128],
                rhs=wt8c[:, 0:2, :cw],
                start=True, stop=False,
                perf_mode=mybir.MatmulPerfMode.DoubleRow)
            nc.tensor.matmul(
                out=ps[:, :cw],
                lhsT=et_s[:, 2:4, b * 128:(b + 1) * 128],
                rhs=wt8c[:, 2:4, :cw],
                start=False, stop=True,
                perf_mode=mybir.MatmulPerfMode.DoubleRow)
            if b in DVE_B:
                nc.vector.tensor_scalar(
                    out=ps[:, :cw].bitcast(I32), in0=ps[:, :cw],
                    scalar1=SCHRA_A, scalar2=SCHRA_B,
                    op0=mybir.AluOpType.mult,
                    op1=mybir.AluOpType.add)
                nc.vector.tensor_reduce(
                    out=zcols[:, b, ct:ct + 1], in_=ps[:, :cw],
                    axis=mybir.AxisListType.X,
                    op=mybir.AluOpType.add)
            else:
                nc.scalar.activation(
                    out=ps[:, :cw], in_=ps[:, :cw],
                    func=mybir.ActivationFunctionType.Exp,
                    scale=S, accum_out=zcols[:, b, ct:ct + 1])

        # interleave the small fp32 target path into early chunks
        if 1 <= ct <= NBT:
            tgt_dot(ct - 1)
        elif ct == NBT + 1:
            tgt_phi()

    # ---- combine partial Z, allgather, final loss ----
    zloc = tiny.tile([128, NBT], F32, name="zloc")
    nc.vector.tensor_reduce(out=zloc, in_=zcols,
                            axis=mybir.AxisListType.X,
                            op=mybir.AluOpType.add)
    nc.sync.dma_start(out=zc_in[:, :], in_=zloc)
    nc.gpsimd.collective_compute(
        "AllGather", mybir.AluOpType.bypass,
        replica_groups=[list(range(N_CORES))],
        ins=[zc_in[:, :]], outs=[zc_out[:, :]])
    # one gather DMA: zg[p, g, b] = zc_out[g*128 + p, b]
    zco = zc_out[:, :]
    zg = tiny.tile([128, N_CORES, NBT], F32, name="zg")
    nc.sync.dma_start(
        out=zg,
        in_=bass.AP(tensor=zco.tensor, offset=zco.offset,
                    ap=[[NBT, 128], [128 * NBT, N_CORES], [1, NBT]]))
    zfull = tiny.tile([128, NBT], F32, name="zfull")
    nc.vector.tensor_reduce(out=zfull, in_=zg.rearrange("p g b -> p b g"),
                            axis=mybir.AxisListType.X,
                            op=mybir.AluOpType.add)
    # Zmod = Z - exp(S cos_t) + exp(S phi);  nll = ln(Zmod) - S phi
    nc.vector.tensor_sub(zfull, zfull, ect)
    nc.vector.tensor_add(zfull, zfull, ept)
    lg = tiny.tile([128, NBT], F32, name="lg")
    nc.scalar.activation(out=lg, in_=zfull,
                         func=mybir.ActivationFunctionType.Ln)
    nll = tiny.tile([128, NBT], F32, name="nll")
    nc.vector.tensor_sub(nll, lg, st)
    nll1 = tiny.tile([128, 1], F32, name="nll1")
    nc.vector.tensor_reduce(out=nll1, in_=nll,
                            axis=mybir.AxisListType.X,
                            op=mybir.AluOpType.add)
    nllr = tiny.tile([128, 1], F32, name="nllr")
    nc.gpsimd.partition_all_reduce(nllr[:, :], nll1[:, :], 128,
                                   bass_isa.ReduceOp.add)
    res = tiny.tile([1, 1], F32, name="res")
    nc.scalar.mul(out=res, in_=nllr[0:1, 0:1], mul=1.0 / B)
    nc.sync.dma_start(out=out_ext[:, :], in_=res)


def _declare_params(nc):
    wt = nc.declare_dram_parameter("wt", [D, CS], FP8, isOutput=False)
    et = nc.declare_dram_parameter("et", [D, B], FP8, isOutput=False)
    er = nc.declare_dram_parameter("er", [B, D], F32, isOutput=False)
    wlab = nc.declare_dram_parameter("wlab", [B, D], F32, isOutput=False)
    out_ext = nc.declare_dram_parameter("out", [1, 1], F32, isOutput=True)
    return (wt, et, er, wlab, out_ext)


def _make_pools(tc, bufs_mult=1):
    return (
        tc.tile_pool(name="singles", bufs=bufs_mult),
        tc.tile_pool(name="work", bufs=4),
        tc.tile_pool(name="wtp", bufs=3),
        tc.tile_pool(name="tiny", bufs=bufs_mult),
        tc.tile_pool(name="psum", bufs=8, space="PSUM"),
    )


def _build(finalize=True):
    nc = bacc.Bacc(num_devices=N_CORES)
    params = _declare_params(nc)
    zc_in = nc.dram_tensor("zc_in", [128, NBT], F32)
    zc_out = nc.dram_tensor("zc_out", [128 * N_CORES, NBT], F32,
                            addr_space="Shared")

    with tile.TileContext(nc) as tc:
        p0, p1, p2, p3, p4 = _make_pools(tc)
        with p0 as singles, p1 as work, p2 as wtp, p3 as tiny, p4 as psump:
            _emit_body(nc, (singles, work, wtp, tiny, psump), params,
                       zc_in, zc_out)

    if finalize:
        nc.finalize()
    return nc


def _get_nc():
    if not _NC_CACHE:
        _NC_CACHE.append(_build())
    return _NC_CACHE[0]


def _prep_inputs(embeddings, labels, weight):
    e = np.asarray(embeddings, dtype=np.float32)
    w = np.asarray(weight, dtype=np.float32)
    lab = np.asarray(labels).astype(np.int64)

    # host-side row L2 normalization (dtype/layout prep for the device matmul)
    en = e / np.maximum(np.sqrt((e * e).sum(axis=1, keepdims=True)), 1e-12)
    wn = w / np.maximum(np.sqrt((w * w).sum(axis=1, keepdims=True)), 1e-12)

    wlab_np = np.ascontiguousarray(wn[lab])                         # [B, D] f32
    et8 = np.ascontiguousarray(en.T).astype(ml_dtypes.float8_e4m3)  # [D, B]
    wt8_full = wn.T.astype(ml_dtypes.float8_e4m3)                   # [D, C]

    in_maps = []
    for i in range(N_CORES):
        sl = slice(CS * i, CS * (i + 1))
        in_maps.append({
            "wt": np.ascontiguousarray(wt8_full[:, sl]),
            "et": et8,
            "er": en,
            "wlab": wlab_np,
        })
    return in_maps


def kernel(embeddings, labels, weight):
    in_maps = _prep_inputs(embeddings, labels, weight)
    nc = _get_nc()
    res = run_bass_kernel_spmd(nc, in_maps, list(range(N_CORES)))
    out = np.asarray(res.results[0]["out"], dtype=np.float32).reshape(())
    return out
